# revision 50
# baseline (speedup 1.0000x reference)
"""Trainium2 Bass kernel for a 2-layer directed GCN (PyG GCNConv semantics).

Strategy (8-core SPMD, 1D node sharding):
  - Layer 1 dest-partitioned: edges grouped by (src chunk, 4-dtile dest group)
    with per-dtile spans padded to a cross-core-uniform width so one SPMD
    program serves all cores. Gathers read a replicated bf16 x table.
    Self-loops are dense diagonal tiles (128 consecutive sources -> the
    diagonal of a dest tile) fed by plain DMA copies from a per-core x slab.
  - Flipped aggregation: matmul(lhsT=gathered_msgs[e,f], rhs=S[e,d]) keeps
    PSUM as agg^T[f, d], so the output matmul (lhsT=agg^T, rhs=W) needs no
    transpose. S[e,d] = (iota==col)*nrm built by one fused DVE tensor_scalar.
  - Layer 2 source-partitioned: each core aggregates h[row] for its OWN h
    slab over edges grouped by global dest tile, evicts bf16 partial sums
    feature-major into a [8*128, 12544] buffer, and a ReduceScatter(add)
    (chunked 4x to overlap with compute) delivers each core its reduced
    dest slab. No AllGather anywhere. The layer-2 self term dinv^2*h is
    applied in the tail via a pre-transposed (h*dinv^2)^T matmul.
"""

import contextlib

import ml_dtypes
import numpy as np

import concourse.bacc as bacc
import concourse.mybir as mybir
import concourse.tile as tile
from concourse.bass_utils import run_bass_kernel_spmd
from concourse.library_config import mlp

N_NODES = 100000
D = 128
N_CORES = 8
NPC = N_NODES // N_CORES          # 12500 nodes per core
TPC = (NPC + 127) // 128          # 98 dest tiles per core
PAD_NPC = TPC * 128               # 12544 padded nodes per core
N_PAD = N_CORES * PAD_NPC         # 100352 padded table rows
NCHUNK = 4
CHUNK_SIZES = (27776, 27392, 28416, 16768)
CHUNK_LO = (0, 27776, 55168, 83584)
assert sum(CHUNK_SIZES) == N_PAD
GW = 4                            # dtiles per layer-1 dest group
NG = (TPC + GW - 1) // GW         # 25 groups (last has 2 dtiles)
GW2 = 2                           # dtiles per layer-2 cell (PSUM banks)
NG2 = (TPC + GW2 - 1) // GW2      # 49 layer-2 groups
RS_CHUNKS = (16, 16, 15, 2)       # RS chunk boundaries, in layer-2 groups
assert sum(RS_CHUNKS) == NG2

F32 = mybir.dt.float32
BF16 = mybir.dt.bfloat16
I16 = mybir.dt.int16
NPBF = ml_dtypes.bfloat16


def _group_dtiles(g):
    return list(range(g * GW, min((g + 1) * GW, TPC)))


def _group_dtiles2(g):
    return list(range(g * GW2, min((g + 1) * GW2, TPC)))


def _pad64(v):
    return -(-int(v) // 64) * 64


def _cell_tiles(widths):
    """Padded spans (64-aligned) for one cell -> (ntiles, per-tile segs).

    widths: list of (dtile, raw_width). Segments are [0,64)/[64,128)/[0,128)
    pieces, legal PE base partitions."""
    bounds = []
    acc = 0
    for d, w in widths:
        pw = _pad64(w)
        if pw:
            bounds.append((d, acc, acc + pw))
        acc += pw
    seg = -(-acc // 128) if acc else 0
    tiles = []
    for ti in range(seg):
        lo_t, hi_t = ti * 128, ti * 128 + 128
        segs = []
        for d, lo, hi in bounds:
            a, b = max(lo, lo_t), min(hi, hi_t)
            if a < b:
                segs.append((d, a - lo_t, b - lo_t))
        tiles.append(segs)
    return seg, tiles


def _plan_layer1(u1):
    """u1[c][d]: uniform per-(chunk,dtile) edge span widths.

    Returns (slots, plan, self_base) where plan[g][c] = (tile_base, seg,
    tiles) and tiles = per-tile segment lists [(dtile, lo, hi)];
    self_base[g] = slot-tile index of group g's self tiles.
    """
    plan = []
    self_base = []
    t = 0
    for g in range(NG):
        dl = _group_dtiles(g)
        per_chunk = []
        for c in range(NCHUNK):
            seg, tiles = _cell_tiles([(d, u1[c][d]) for d in dl])
            per_chunk.append((t, seg, tiles))
            t += seg
        plan.append(per_chunk)
        self_base.append(t)
        t += len(dl)
    return t, plan, self_base


def _plan_layer2(u2):
    """u2[dc][d]: uniform per-(dest core, dtile) span widths.

    plan[g] = (call_base, call_seg, cells) with cells[dc] = (tile_base,
    ntiles, tiles-with-segments)."""
    plan = []
    t = 0
    for g in range(NG2):
        dl = _group_dtiles2(g)
        cells = []
        call_base = t
        for dc in range(N_CORES):
            seg, tiles = _cell_tiles([(d, u2[dc][d]) for d in dl])
            cells.append((t, seg, tiles))
            t += seg
        plan.append((call_base, t - call_base, cells))
    return t, plan


def _build_nc(u1_key, u2_key):
    u1 = np.array(u1_key, dtype=np.int64).reshape(NCHUNK, TPC)
    u2 = np.array(u2_key, dtype=np.int64).reshape(N_CORES, TPC)
    NS1, plan1, self1 = _plan_layer1(u1)     # slot tiles incl self tiles
    NT2, plan2 = _plan_layer2(u2)
    max_seg1 = max(seg for pc in plan1 for (_, seg, _) in pc) or 1
    max_seg2 = max(seg for (_, seg, _) in plan2) or 1
    maxseg = max(max_seg1, max_seg2)


    nc = bacc.Bacc("TRN2", target_bir_lowering=False)

    x_tab = nc.dram_tensor("x_tab", [N_PAD, D], BF16, kind="ExternalInput")
    # own x slab pre-permuted on host: column d*D+f holds x[d*128+p, f]
    x_self = nc.dram_tensor("x_self", [128, TPC * D], BF16,
                            kind="ExternalInput")
    gix1 = nc.dram_tensor("gix1", [128, NS1 * 8], I16, kind="ExternalInput")
    colw1 = nc.dram_tensor("colw1", [128, NS1], F32, kind="ExternalInput")
    nrmw1 = nc.dram_tensor("nrmw1", [128, NS1], F32, kind="ExternalInput")
    gix2 = nc.dram_tensor("gix2", [128, NT2 * 8], I16, kind="ExternalInput")
    colw2 = nc.dram_tensor("colw2", [128, NT2], F32, kind="ExternalInput")
    nrmw2 = nc.dram_tensor("nrmw2", [128, NT2], F32, kind="ExternalInput")
    selfw = nc.dram_tensor("selfw", [128, TPC], F32, kind="ExternalInput")
    w1 = nc.dram_tensor("w1", [D, D], BF16, kind="ExternalInput")
    b1 = nc.dram_tensor("b1", [1, D], BF16, kind="ExternalInput")
    w2 = nc.dram_tensor("w2", [D, D], BF16, kind="ExternalInput")
    b2 = nc.dram_tensor("b2", [1, D], BF16, kind="ExternalInput")
    ident = nc.dram_tensor("ident", [128, 128], F32, kind="ExternalInput")
    iota = nc.dram_tensor("iota", [128, 128], BF16, kind="ExternalInput")
    ones = nc.dram_tensor("ones", [1, D], BF16, kind="ExternalInput")
    out_slab = nc.dram_tensor("out_slab", [NPC, D], F32, kind="ExternalOutput")

    with tile.TileContext(nc) as tc:
        nc.gpsimd.load_library(mlp)
        with (
            tc.tile_pool(name="const", bufs=1) as constp,
            tc.tile_pool(name="gbuf1", bufs=3) as gbuf1p,
            tc.tile_pool(name="gbuf2", bufs=4) as gbuf2p,
            tc.tile_pool(name="sbld", bufs=8) as sbldp,
            tc.tile_pool(name="stage", bufs=4) as stagep,
            tc.tile_pool(name="stage2", bufs=2) as stage2p,
            tc.tile_pool(name="hkeep", bufs=1) as hkeepp,
            tc.tile_pool(name="dram", bufs=1, space="DRAM") as dramp,
        ):
            gix1_s = constp.tile([128, NS1 * 8], I16, tag="gix1")
            colw1_s = constp.tile([128, NS1], F32, tag="colw1")
            nrmw1_s = constp.tile([128, NS1], F32, tag="nrmw1")
            gix2_s = constp.tile([128, NT2 * 8], I16, tag="gix2")
            colw2_s = constp.tile([128, NT2], F32, tag="colw2")
            nrmw2_s = constp.tile([128, NT2], F32, tag="nrmw2")
            selfw_s = constp.tile([128, TPC], F32, tag="selfw")
            w1_s = constp.tile([D, D], BF16, tag="w1")
            b1_s = constp.tile([1, D], BF16, tag="b1")
            w2_s = constp.tile([D, D], BF16, tag="w2")
            b2_s = constp.tile([1, D], BF16, tag="b2")
            id_s = constp.tile([128, 128], F32, tag="ident")
            iota_s = constp.tile([128, 128], BF16, tag="iota")
            ones_s = constp.tile([1, D], BF16, tag="ones")
            # layer-1 constants first so layer 1 can start ASAP
            for dst, srct in ((gix1_s, gix1), (colw1_s, colw1),
                              (nrmw1_s, nrmw1), (iota_s, iota),
                              (w1_s, w1), (b1_s, b1), (id_s, ident),
                              (ones_s, ones), (selfw_s, selfw),
                              (gix2_s, gix2), (colw2_s, colw2),
                              (nrmw2_s, nrmw2), (w2_s, w2), (b2_s, b2)):
                nc.sync.dma_start(dst[:], srct[:])

            h_loc = dramp.tile([PAD_NPC, D], BF16, tag="h_loc")
            # per-RS-chunk partial/reduced buffers (collective ins must be
            # contiguous whole tensors)
            chunk_cols = []
            g_hi = 0
            for cgroups in RS_CHUNKS:
                g_lo, g_hi = g_hi, g_hi + cgroups
                lo = g_lo * GW2 * 128
                hi = g_hi * GW2 * 128 if g_hi < NG2 else PAD_NPC
                chunk_cols.append((lo, hi))
            partials = [dramp.tile([N_CORES * 128, hi - lo], BF16,
                                   tag=f"partial{i}", name=f"partial{i}")
                        for i, (lo, hi) in enumerate(chunk_cols)]
            reduceds = [dramp.tile([128, hi - lo], BF16,
                                   tag=f"reduced{i}", name=f"reduced{i}")
                        for i, (lo, hi) in enumerate(chunk_cols)]

            # hsT[f, d] = (h * dinv^2)^T kept resident for the L2 self term
            hsT_sb = hkeepp.tile([128, TPC * 128], BF16, tag="hsT")

            def build_s(tile_idx, colw_s, nrmw_s):
                s_t = sbldp.tile([128, 128], BF16, tag="sbld")
                nc.vector.tensor_scalar(
                    s_t[:], iota_s[:],
                    colw_s[:, tile_idx:tile_idx + 1],
                    nrmw_s[:, tile_idx:tile_idx + 1],
                    mybir.AluOpType.is_equal, mybir.AluOpType.mult)
                return s_t

            # ---------------- Layer 1 ----------------
            l1pools = contextlib.ExitStack()
            psagp = l1pools.enter_context(
                tc.tile_pool(name="psag1", bufs=4, space="PSUM"))
            pstp = l1pools.enter_context(
                tc.tile_pool(name="pst1", bufs=2, space="PSUM"))
            psop = l1pools.enter_context(
                tc.tile_pool(name="pso1", bufs=2, space="PSUM"))
            for g in range(NG):
                dl = _group_dtiles(g)
                psmap = {d: psagp.tile([128, 128], F32, tag="psag",
                                       name=f"ps1_{g}_{d}") for d in dl}
                started = {d: False for d in dl}
                nmm = {d: 0 for d in dl}
                total_mm = {d: 1 for d in dl}       # +1 for the self tile
                for c in range(NCHUNK):
                    _, _, tiles = plan1[g][c]
                    for segs in tiles:
                        for d, _, _ in segs:
                            total_mm[d] += 1

                def psd(d, psmap=psmap):
                    return psmap[d][:]

                for c in range(NCHUNK):
                    t0, seg, tiles = plan1[g][c]
                    if seg == 0:
                        continue
                    gt = gbuf1p.tile([128, max_seg1, 128], BF16, tag="gbuf1")
                    nc.gpsimd.dma_gather(
                        gt[:, :seg, :],
                        x_tab[CHUNK_LO[c]:CHUNK_LO[c] + CHUNK_SIZES[c], :],
                        gix1_s[:, t0 * 8:(t0 + seg) * 8],
                        seg * 128, seg * 128, D, single_packet=False)
                    for ti, segs in enumerate(tiles):
                        s_t = build_s(t0 + ti, colw1_s, nrmw1_s)
                        for d, lo, hi in segs:
                            nmm[d] += 1
                            nc.tensor.matmul(
                                psd(d), gt[lo:hi, ti, :], s_t[lo:hi, :],
                                start=not started[d],
                                stop=nmm[d] == total_mm[d])
                            started[d] = True
                # self tiles: diagonal contribution from own x slab
                sb = stage2p.tile([128, GW * 128], BF16, tag="selfb")
                nc.sync.dma_start(
                    sb[:, :len(dl) * 128],
                    x_self[:, g * GW * D:(g * GW + len(dl)) * D])
                for j, d in enumerate(dl):
                    s_t = build_s(self1[g] + j, colw1_s, nrmw1_s)
                    nmm[d] += 1
                    nc.tensor.matmul(psd(d), sb[:, j * 128:(j + 1) * 128],
                                     s_t[:],
                                     start=not started[d],
                                     stop=nmm[d] == total_mm[d])
                    started[d] = True
                # tail: aggT -> h (batched store), hsT
                h_stage = stage2p.tile([128, GW * 128], BF16, tag="hstage")
                for j, d in enumerate(dl):
                    aggt = stagep.tile([128, 128], BF16, tag="aggt")
                    nc.scalar.activation(aggt[:], psd(d),
                                         mybir.ActivationFunctionType.Copy)
                    ps_h = psop.tile([128, 128], F32, tag="pso")
                    nc.tensor.matmul(ps_h[:], aggt[:], w1_s[:],
                                     start=True, stop=False)
                    nc.tensor.matmul(ps_h[:], ones_s[:], b1_s[:],
                                     start=False, stop=True)
                    h_sb = h_stage[:, j * 128:(j + 1) * 128]
                    nc.scalar.activation(
                        h_sb, ps_h[:], mybir.ActivationFunctionType.Relu)
                    hs = stagep.tile([128, 128], F32, tag="hs")
                    nc.vector.tensor_scalar(
                        hs[:], h_sb, selfw_s[:, d:d + 1], None,
                        mybir.AluOpType.mult)
                    ps_t = pstp.tile([128, 128], F32, tag="pst")
                    nc.tensor.transpose(ps_t[:], hs[:], id_s[:])
                    nc.scalar.activation(
                        hsT_sb[:, d * 128:(d + 1) * 128], ps_t[:],
                        mybir.ActivationFunctionType.Copy)
                hl = h_loc[g * GW * 128:(g * GW + len(dl)) * 128, :]
                hl_p = type(hl)(hl.tensor, hl.offset,
                                [[D, 128], [128 * D, len(dl)], [1, D]])
                nc.sync.dma_start(hl_p, h_stage[:, :len(dl) * 128])
            l1pools.close()

            # ---------------- Layer 2 aggregation + chunked RS ------------
            l2pools = contextlib.ExitStack()
            psag2p = l2pools.enter_context(
                tc.tile_pool(name="psag2", bufs=6, space="PSUM"))
            pso2p = l2pools.enter_context(
                tc.tile_pool(name="pso2", bufs=2, space="PSUM"))
            def tail_chunk(ci):
                col0 = chunk_cols[ci][0]
                reduced = reduceds[ci]
                g_lo = sum(RS_CHUNKS[:ci])
                g_end = g_lo + RS_CHUNKS[ci]
                for g in range(g_lo, g_end):
                    if (g - g_lo) % 4 == 0:
                        nload = min(4, g_end - g)
                        agg2q = stage2p.tile([128, 4 * GW2 * 128], BF16,
                                             tag="agg2")
                        qbase = g * GW2 * 128 - col0
                        qw = 0
                        for gg in range(g, g + nload):
                            qw += len(_group_dtiles2(gg)) * 128
                        nc.sync.dma_start(agg2q[:, :qw],
                                           reduced[:, qbase:qbase + qw])
                        qoff = 0
                    dl = _group_dtiles2(g)
                    agg2 = agg2q[:, qoff:qoff + len(dl) * 128]
                    qoff += len(dl) * 128
                    for j, d in enumerate(dl):
                        ps_o = pso2p.tile([128, 128], F32, tag="pso2")
                        nc.tensor.matmul(
                            ps_o[:], agg2[:, j * 128:(j + 1) * 128], w2_s[:],
                            start=True, stop=False)
                        nc.tensor.matmul(
                            ps_o[:], hsT_sb[:, d * 128:(d + 1) * 128],
                            w2_s[:], start=False, stop=False)
                        nc.tensor.matmul(ps_o[:], ones_s[:], b2_s[:],
                                         start=False, stop=True)
                        o_sb = stagep.tile([128, 128], F32, tag="osb")
                        nc.scalar.activation(
                            o_sb[:], ps_o[:],
                            mybir.ActivationFunctionType.Copy)
                        lo = d * 128
                        hi = min(lo + 128, NPC)
                        nc.sync.dma_start(out_slab[lo:hi, :],
                                           o_sb[:hi - lo, :])

            def emit_rs(ci):
                nc.gpsimd.collective_compute(
                    "ReduceScatter", mybir.AluOpType.add,
                    replica_groups=[list(range(N_CORES))],
                    ins=[partials[ci].opt()], outs=[reduceds[ci].opt()])

            g_hi = 0
            for ci, cgroups in enumerate(RS_CHUNKS):
                g_lo, g_hi = g_hi, g_hi + cgroups
                col0 = chunk_cols[ci][0]
                partial = partials[ci]
                # pair consecutive groups per dc so partial stores batch
                pairs = []
                gg = g_lo
                while gg < g_hi:
                    pairs.append((gg, min(gg + 2, g_hi)))
                    gg += 2
                for pi, (p_lo, p_hi) in enumerate(pairs):
                    if pi == min(4, len(pairs) - 1):
                        # issue the previous chunk's RS now: the first pair's
                        # gathers are already queued on Pool, bridging the
                        # RS dep-wait; its input stores completed by now
                        if ci >= 1:
                            emit_rs(ci - 1)
                        if ci >= 2:
                            tail_chunk(ci - 2)
                    stgs = {}
                    for dc in range(N_CORES):
                        w = sum(len(_group_dtiles2(x)) * 128
                                for x in range(p_lo, p_hi))
                        stg_t = stage2p.tile([128, 2 * GW2 * 128], BF16,
                                             tag=f"pstg{dc}",
                                             name=f"pstg_{dc}")
                        stgs[dc] = (stg_t, w)
                    for g in range(p_lo, p_hi):
                        dl = _group_dtiles2(g)
                        call_base, call_seg, cells = plan2[g]
                        gt = gbuf2p.tile([128, max_seg2, 128], BF16,
                                         tag="gbuf2")
                        if call_seg:
                            nc.gpsimd.dma_gather(
                                gt[:, :call_seg, :], h_loc[:],
                                gix2_s[:,
                                       call_base * 8:(call_base + call_seg) * 8],
                                call_seg * 128, call_seg * 128, D,
                                single_packet=False)
                        goff = (g - p_lo) * GW2 * 128
                        for dc in range(N_CORES):
                            t0, seg, tiles = cells[dc]
                            stg = stgs[dc][0]
                            psmap = {d: psag2p.tile([128, 128], F32,
                                                    tag="psag2",
                                                    name=f"ps2_{g}_{dc}_{d}")
                                     for d in dl}
                            started = {d: False for d in dl}
                            nmm = {d: 0 for d in dl}
                            total_mm = {d: 0 for d in dl}
                            for segs in tiles:
                                for d, _, _ in segs:
                                    total_mm[d] += 1
                            for ti, segs in enumerate(tiles):
                                s_t = build_s(t0 + ti, colw2_s, nrmw2_s)
                                for d, lo, hi in segs:
                                    nmm[d] += 1
                                    nc.tensor.matmul(
                                        psmap[d][:],
                                        gt[lo:hi, t0 - call_base + ti, :],
                                        s_t[lo:hi, :],
                                        start=not started[d],
                                        stop=nmm[d] == total_mm[d])
                                    started[d] = True
                            for j, d in enumerate(dl):
                                dst = stg[:, goff + j * 128:
                                          goff + (j + 1) * 128]
                                if total_mm[d] == 0:
                                    nc.vector.memset(dst, 0.0)
                                else:
                                    nc.scalar.activation(
                                        dst, psmap[d][:],
                                        mybir.ActivationFunctionType.Copy)
                    base = p_lo * GW2 * 128 - col0
                    for dc in range(N_CORES):
                        stg, w = stgs[dc]
                        eng = nc.sync if dc % 2 == 0 else nc.scalar
                        eng.dma_start(
                            partial[dc * 128:(dc + 1) * 128, base:base + w],
                            stg[:, :w])
            nch = len(RS_CHUNKS)
            tail_chunk(nch - 2)
            emit_rs(nch - 1)
            tail_chunk(nch - 1)
            l2pools.close()

    nc.compile()
    return nc


def _preprocess(x, edge_index, edge_weight):
    """Host-side graph preprocessing -> uniform structure + per-core inputs."""
    row = np.asarray(edge_index[0], dtype=np.int64)
    col = np.asarray(edge_index[1], dtype=np.int64)
    ew = np.asarray(edge_weight, dtype=np.float32)

    deg = np.bincount(col, weights=ew.astype(np.float64), minlength=N_NODES)
    deg = (deg + 1.0).astype(np.float32)
    dinv = (1.0 / np.sqrt(deg)).astype(np.float32)
    nrm = (dinv[row] * ew * dinv[col]).astype(np.float32)
    selfw_n = (dinv * dinv).astype(np.float32)

    score = row // NPC
    dcore = col // NPC
    dtile = (col - dcore * NPC) >> 7
    colloc = ((col - dcore * NPC) & 127).astype(np.float32)
    pad_row = (score * PAD_NPC + (row - score * NPC)).astype(np.int64)
    chunk = np.searchsorted(np.asarray(CHUNK_LO), pad_row, side="right") - 1
    grp = dtile // GW

    # ---- layer 1 (keyed by dest core) ----
    key1 = ((dcore * NCHUNK + chunk) * TPC + dtile)
    cnt1 = np.bincount(key1, minlength=N_CORES * NCHUNK * TPC)
    u1 = cnt1.reshape(N_CORES, NCHUNK, TPC).max(axis=0)      # [NCHUNK, TPC]

    NS1, plan1, self1 = _plan_layer1(u1)
    # slot position of (chunk, dtile) span starts (64-padded spans)
    base1 = np.zeros((NCHUNK, TPC), np.int64)
    for g in range(NG):
        for c in range(NCHUNK):
            t0, _, _ = plan1[g][c]
            acc = t0 * 128
            for d in _group_dtiles(g):
                base1[c, d] = acc
                acc += _pad64(u1[c, d])

    order = np.argsort(key1, kind="stable")
    ks = key1[order]
    gstart = np.concatenate(
        [[0], np.cumsum(np.bincount(ks, minlength=key1.max() + 1))[:-1]])
    rank = np.arange(len(ks)) - gstart[ks]

    gix1f = np.zeros((N_CORES, NS1 * 128), np.int16)
    colw1f = np.zeros((N_CORES, NS1 * 128), np.float32)
    nrmw1f = np.zeros((N_CORES, NS1 * 128), np.float32)
    pos = base1[chunk[order], dtile[order]] + rank
    cidx = dcore[order]
    gix1f[cidx, pos] = (pad_row[order]
                        - np.asarray(CHUNK_LO)[chunk[order]]).astype(np.int16)
    colw1f[cidx, pos] = colloc[order]
    nrmw1f[cidx, pos] = nrm[order]
    # self tiles (same for every core's slot space; values differ per core)
    idx_all = np.arange(N_NODES)
    c_all = idx_all // NPC
    selfw_pad = np.zeros(N_CORES * PAD_NPC, np.float32)
    selfw_pad[c_all * PAD_NPC + (idx_all - c_all * NPC)] = selfw_n
    selfw_pc = selfw_pad.reshape(N_CORES, PAD_NPC)
    for g in range(NG):
        for j, d in enumerate(_group_dtiles(g)):
            s = (self1[g] + j) * 128
            colw1f[:, s:s + 128] = np.arange(128, dtype=np.float32)
            nrmw1f[:, s:s + 128] = selfw_pc[:, d * 128:(d + 1) * 128]

    # ---- layer 2 (keyed by source core) ----
    key2 = ((score * N_CORES + dcore) * TPC + dtile)
    cnt2 = np.bincount(key2, minlength=N_CORES * N_CORES * TPC)
    u2 = cnt2.reshape(N_CORES, N_CORES, TPC).max(axis=0)     # [dcore, TPC]

    NT2, plan2 = _plan_layer2(u2)
    base2 = np.zeros((N_CORES, TPC), np.int64)
    for g in range(NG2):
        _, _, cells = plan2[g]
        for dc in range(N_CORES):
            t0, _, _ = cells[dc]
            acc = t0 * 128
            for d in _group_dtiles2(g):
                base2[dc, d] = acc
                acc += _pad64(u2[dc, d])

    order2 = np.argsort(key2, kind="stable")
    ks2 = key2[order2]
    gstart2 = np.concatenate(
        [[0], np.cumsum(np.bincount(ks2, minlength=key2.max() + 1))[:-1]])
    rank2 = np.arange(len(ks2)) - gstart2[ks2]

    gix2f = np.zeros((N_CORES, NT2 * 128), np.int16)
    colw2f = np.zeros((N_CORES, NT2 * 128), np.float32)
    nrmw2f = np.zeros((N_CORES, NT2 * 128), np.float32)
    pos2 = base2[dcore[order2], dtile[order2]] + rank2
    cidx2 = score[order2]
    gix2f[cidx2, pos2] = (row[order2] - score[order2] * NPC).astype(np.int16)
    colw2f[cidx2, pos2] = colloc[order2]
    nrmw2f[cidx2, pos2] = nrm[order2]

    def wrap_gix(gixf, nt):
        g = gixf.reshape(N_CORES, nt * 8, 16).transpose(0, 2, 1)
        return np.ascontiguousarray(np.tile(g, (1, 8, 1)))

    def wrap_col(f, nt):
        return np.ascontiguousarray(
            f.reshape(N_CORES, nt, 128).transpose(0, 2, 1))

    gix1w = wrap_gix(gix1f, NS1)
    colw1w = wrap_col(colw1f, NS1)
    nrmw1w = wrap_col(nrmw1f, NS1)
    gix2w = wrap_gix(gix2f, NT2)
    colw2w = wrap_col(colw2f, NT2)
    nrmw2w = wrap_col(nrmw2f, NT2)

    selfw_t = np.ascontiguousarray(
        selfw_pad.reshape(N_CORES, TPC, 128).transpose(0, 2, 1))

    x = np.asarray(x, dtype=np.float32)
    x_tab = np.zeros((N_PAD, D), NPBF)
    x_tab.reshape(N_CORES, PAD_NPC, D)[:, :NPC, :] = \
        x.reshape(N_CORES, NPC, D).astype(NPBF)
    # per-core x slab permuted to [row-within-dtile, dtile*D + f]
    x_selfs = np.ascontiguousarray(
        x_tab.reshape(N_CORES, TPC, 128, D).transpose(0, 2, 1, 3)
        .reshape(N_CORES, 128, TPC * D))

    u1_key = tuple(int(v) for v in u1.reshape(-1))
    u2_key = tuple(int(v) for v in u2.reshape(-1))
    return (u1_key, u2_key, gix1w, colw1w, nrmw1w, gix2w, colw2w, nrmw2w,
            selfw_t, x_tab, x_selfs)


_NC_CACHE: dict = {}


def kernel(x, edge_index, edge_weight, W1, b1, W2, b2):
    (u1_key, u2_key, gix1w, colw1w, nrmw1w, gix2w, colw2w, nrmw2w,
     selfw_t, x_tab, x_selfs) = _preprocess(x, edge_index, edge_weight)

    ck = (u1_key, u2_key)
    if ck not in _NC_CACHE:
        _NC_CACHE[ck] = _build_nc(u1_key, u2_key)
    nc = _NC_CACHE[ck]

    w1_np = np.ascontiguousarray(np.asarray(W1, dtype=np.float32).astype(NPBF))
    w2_np = np.ascontiguousarray(np.asarray(W2, dtype=np.float32).astype(NPBF))
    b1_np = np.asarray(b1, dtype=np.float32).astype(NPBF).reshape(1, D)
    b2_np = np.asarray(b2, dtype=np.float32).astype(NPBF).reshape(1, D)
    ident = np.eye(128, dtype=np.float32)
    iota = np.tile(np.arange(128), (128, 1)).astype(NPBF)
    ones = np.ones((1, D), NPBF)

    in_maps = []
    for c in range(N_CORES):
        in_maps.append({
            "x_tab": x_tab, "x_self": x_selfs[c],
            "gix1": gix1w[c], "colw1": colw1w[c], "nrmw1": nrmw1w[c],
            "gix2": gix2w[c], "colw2": colw2w[c], "nrmw2": nrmw2w[c],
            "selfw": selfw_t[c],
            "w1": w1_np, "b1": b1_np, "w2": w2_np, "b2": b2_np,
            "ident": ident, "iota": iota, "ones": ones,
        })

    res = run_bass_kernel_spmd(nc, in_maps, core_ids=list(range(N_CORES)))
    out = np.concatenate([res.results[c]["out_slab"] for c in range(N_CORES)],
                         axis=0)
    return out


# revision 51
# speedup vs baseline: 1.0002x; 1.0002x over previous
"""Trainium2 Bass kernel for a 2-layer directed GCN (PyG GCNConv semantics).

Strategy (8-core SPMD, 1D node sharding):
  - Layer 1 dest-partitioned: edges grouped by (src chunk, 4-dtile dest group)
    with per-dtile spans padded to a cross-core-uniform width so one SPMD
    program serves all cores. Gathers read a replicated bf16 x table.
    Self-loops are dense diagonal tiles (128 consecutive sources -> the
    diagonal of a dest tile) fed by plain DMA copies from a per-core x slab.
  - Flipped aggregation: matmul(lhsT=gathered_msgs[e,f], rhs=S[e,d]) keeps
    PSUM as agg^T[f, d], so the output matmul (lhsT=agg^T, rhs=W) needs no
    transpose. S[e,d] = (iota==col)*nrm built by one fused DVE tensor_scalar.
  - Layer 2 source-partitioned: each core aggregates h[row] for its OWN h
    slab over edges grouped by global dest tile, evicts bf16 partial sums
    feature-major into a [8*128, 12544] buffer, and a ReduceScatter(add)
    (chunked 4x to overlap with compute) delivers each core its reduced
    dest slab. No AllGather anywhere. The layer-2 self term dinv^2*h is
    applied in the tail via a pre-transposed (h*dinv^2)^T matmul.
"""

import contextlib

import ml_dtypes
import numpy as np

import concourse.bacc as bacc
import concourse.mybir as mybir
import concourse.tile as tile
from concourse.bass_utils import run_bass_kernel_spmd
from concourse.library_config import mlp

N_NODES = 100000
D = 128
N_CORES = 8
NPC = N_NODES // N_CORES          # 12500 nodes per core
TPC = (NPC + 127) // 128          # 98 dest tiles per core
PAD_NPC = TPC * 128               # 12544 padded nodes per core
N_PAD = N_CORES * PAD_NPC         # 100352 padded table rows
NCHUNK = 4
CHUNK_SIZES = (27776, 27392, 28416, 16768)
CHUNK_LO = (0, 27776, 55168, 83584)
assert sum(CHUNK_SIZES) == N_PAD
GW = 4                            # dtiles per layer-1 dest group
NG = (TPC + GW - 1) // GW         # 25 groups (last has 2 dtiles)
GW2 = 2                           # dtiles per layer-2 cell (PSUM banks)
NG2 = (TPC + GW2 - 1) // GW2      # 49 layer-2 groups
RS_CHUNKS = (16, 16, 15, 2)       # RS chunk boundaries, in layer-2 groups
assert sum(RS_CHUNKS) == NG2

F32 = mybir.dt.float32
BF16 = mybir.dt.bfloat16
I16 = mybir.dt.int16
NPBF = ml_dtypes.bfloat16


def _group_dtiles(g):
    return list(range(g * GW, min((g + 1) * GW, TPC)))


def _group_dtiles2(g):
    return list(range(g * GW2, min((g + 1) * GW2, TPC)))


def _pad64(v):
    return -(-int(v) // 64) * 64


def _cell_tiles(widths):
    """Padded spans (64-aligned) for one cell -> (ntiles, per-tile segs).

    widths: list of (dtile, raw_width). Segments are [0,64)/[64,128)/[0,128)
    pieces, legal PE base partitions."""
    bounds = []
    acc = 0
    for d, w in widths:
        pw = _pad64(w)
        if pw:
            bounds.append((d, acc, acc + pw))
        acc += pw
    seg = -(-acc // 128) if acc else 0
    tiles = []
    for ti in range(seg):
        lo_t, hi_t = ti * 128, ti * 128 + 128
        segs = []
        for d, lo, hi in bounds:
            a, b = max(lo, lo_t), min(hi, hi_t)
            if a < b:
                segs.append((d, a - lo_t, b - lo_t))
        tiles.append(segs)
    return seg, tiles


def _plan_layer1(u1):
    """u1[c][d]: uniform per-(chunk,dtile) edge span widths.

    Returns (slots, plan, self_base) where plan[g][c] = (tile_base, seg,
    tiles) and tiles = per-tile segment lists [(dtile, lo, hi)];
    self_base[g] = slot-tile index of group g's self tiles.
    """
    plan = []
    self_base = []
    t = 0
    for g in range(NG):
        dl = _group_dtiles(g)
        per_chunk = []
        for c in range(NCHUNK):
            seg, tiles = _cell_tiles([(d, u1[c][d]) for d in dl])
            per_chunk.append((t, seg, tiles))
            t += seg
        plan.append(per_chunk)
        self_base.append(t)
        t += len(dl)
    return t, plan, self_base


def _plan_layer2(u2):
    """u2[dc][d]: uniform per-(dest core, dtile) span widths.

    plan[g] = (call_base, call_seg, cells) with cells[dc] = (tile_base,
    ntiles, tiles-with-segments)."""
    plan = []
    t = 0
    for g in range(NG2):
        dl = _group_dtiles2(g)
        cells = []
        call_base = t
        for dc in range(N_CORES):
            seg, tiles = _cell_tiles([(d, u2[dc][d]) for d in dl])
            cells.append((t, seg, tiles))
            t += seg
        plan.append((call_base, t - call_base, cells))
    return t, plan


def _build_nc(u1_key, u2_key):
    u1 = np.array(u1_key, dtype=np.int64).reshape(NCHUNK, TPC)
    u2 = np.array(u2_key, dtype=np.int64).reshape(N_CORES, TPC)
    NS1, plan1, self1 = _plan_layer1(u1)     # slot tiles incl self tiles
    NT2, plan2 = _plan_layer2(u2)
    max_seg1 = max(seg for pc in plan1 for (_, seg, _) in pc) or 1
    max_seg2 = max(seg for (_, seg, _) in plan2) or 1
    maxseg = max(max_seg1, max_seg2)


    nc = bacc.Bacc("TRN2", target_bir_lowering=False)

    x_tab = nc.dram_tensor("x_tab", [N_PAD, D], BF16, kind="ExternalInput")
    # own x slab pre-permuted on host: column d*D+f holds x[d*128+p, f]
    x_self = nc.dram_tensor("x_self", [128, TPC * D], BF16,
                            kind="ExternalInput")
    gix1 = nc.dram_tensor("gix1", [128, NS1 * 8], I16, kind="ExternalInput")
    colw1 = nc.dram_tensor("colw1", [128, NS1], F32, kind="ExternalInput")
    nrmw1 = nc.dram_tensor("nrmw1", [128, NS1], F32, kind="ExternalInput")
    gix2 = nc.dram_tensor("gix2", [128, NT2 * 8], I16, kind="ExternalInput")
    colw2 = nc.dram_tensor("colw2", [128, NT2], F32, kind="ExternalInput")
    nrmw2 = nc.dram_tensor("nrmw2", [128, NT2], F32, kind="ExternalInput")
    selfw = nc.dram_tensor("selfw", [128, TPC], F32, kind="ExternalInput")
    w1 = nc.dram_tensor("w1", [D, D], BF16, kind="ExternalInput")
    b1 = nc.dram_tensor("b1", [1, D], BF16, kind="ExternalInput")
    w2 = nc.dram_tensor("w2", [D, D], BF16, kind="ExternalInput")
    b2 = nc.dram_tensor("b2", [1, D], BF16, kind="ExternalInput")
    ident = nc.dram_tensor("ident", [128, 128], F32, kind="ExternalInput")
    iota = nc.dram_tensor("iota", [128, 128], BF16, kind="ExternalInput")
    ones = nc.dram_tensor("ones", [1, D], BF16, kind="ExternalInput")
    out_slab = nc.dram_tensor("out_slab", [NPC, D], F32, kind="ExternalOutput")

    with tile.TileContext(nc) as tc:
        nc.gpsimd.load_library(mlp)
        with (
            tc.tile_pool(name="const", bufs=1) as constp,
            tc.tile_pool(name="gbuf1", bufs=3) as gbuf1p,
            tc.tile_pool(name="gbuf2", bufs=4) as gbuf2p,
            tc.tile_pool(name="sbld", bufs=8) as sbldp,
            tc.tile_pool(name="stage", bufs=4) as stagep,
            tc.tile_pool(name="stage2", bufs=2) as stage2p,
            tc.tile_pool(name="hkeep", bufs=1) as hkeepp,
            tc.tile_pool(name="dram", bufs=1, space="DRAM") as dramp,
        ):
            gix1_s = constp.tile([128, NS1 * 8], I16, tag="gix1")
            colw1_s = constp.tile([128, NS1], F32, tag="colw1")
            nrmw1_s = constp.tile([128, NS1], F32, tag="nrmw1")
            gix2_s = constp.tile([128, NT2 * 8], I16, tag="gix2")
            colw2_s = constp.tile([128, NT2], F32, tag="colw2")
            nrmw2_s = constp.tile([128, NT2], F32, tag="nrmw2")
            selfw_s = constp.tile([128, TPC], F32, tag="selfw")
            w1_s = constp.tile([D, D], BF16, tag="w1")
            b1_s = constp.tile([1, D], BF16, tag="b1")
            w2_s = constp.tile([D, D], BF16, tag="w2")
            b2_s = constp.tile([1, D], BF16, tag="b2")
            id_s = constp.tile([128, 128], F32, tag="ident")
            iota_s = constp.tile([128, 128], BF16, tag="iota")
            ones_s = constp.tile([1, D], BF16, tag="ones")
            # layer-1 constants first so layer 1 can start ASAP
            for dst, srct in ((gix1_s, gix1), (colw1_s, colw1),
                              (nrmw1_s, nrmw1), (iota_s, iota),
                              (w1_s, w1), (b1_s, b1), (id_s, ident),
                              (ones_s, ones), (selfw_s, selfw),
                              (gix2_s, gix2), (colw2_s, colw2),
                              (nrmw2_s, nrmw2), (w2_s, w2), (b2_s, b2)):
                nc.sync.dma_start(dst[:], srct[:])

            h_loc = dramp.tile([PAD_NPC, D], BF16, tag="h_loc")
            # per-RS-chunk partial/reduced buffers (collective ins must be
            # contiguous whole tensors)
            chunk_cols = []
            g_hi = 0
            for cgroups in RS_CHUNKS:
                g_lo, g_hi = g_hi, g_hi + cgroups
                lo = g_lo * GW2 * 128
                hi = g_hi * GW2 * 128 if g_hi < NG2 else PAD_NPC
                chunk_cols.append((lo, hi))
            partials = [dramp.tile([N_CORES * 128, hi - lo], BF16,
                                   tag=f"partial{i}", name=f"partial{i}")
                        for i, (lo, hi) in enumerate(chunk_cols)]
            reduceds = [dramp.tile([128, hi - lo], BF16,
                                   tag=f"reduced{i}", name=f"reduced{i}")
                        for i, (lo, hi) in enumerate(chunk_cols)]

            # hsT[f, d] = (h * dinv^2)^T kept resident for the L2 self term
            hsT_sb = hkeepp.tile([128, TPC * 128], BF16, tag="hsT")

            def build_s(tile_idx, colw_s, nrmw_s):
                s_t = sbldp.tile([128, 128], BF16, tag="sbld")
                nc.vector.tensor_scalar(
                    s_t[:], iota_s[:],
                    colw_s[:, tile_idx:tile_idx + 1],
                    nrmw_s[:, tile_idx:tile_idx + 1],
                    mybir.AluOpType.is_equal, mybir.AluOpType.mult)
                return s_t

            # ---------------- Layer 1 ----------------
            l1pools = contextlib.ExitStack()
            psagp = l1pools.enter_context(
                tc.tile_pool(name="psag1", bufs=4, space="PSUM"))
            pstp = l1pools.enter_context(
                tc.tile_pool(name="pst1", bufs=2, space="PSUM"))
            psop = l1pools.enter_context(
                tc.tile_pool(name="pso1", bufs=2, space="PSUM"))
            for g in range(NG):
                dl = _group_dtiles(g)
                psmap = {d: psagp.tile([128, 128], F32, tag="psag",
                                       name=f"ps1_{g}_{d}") for d in dl}
                started = {d: False for d in dl}
                nmm = {d: 0 for d in dl}
                total_mm = {d: 1 for d in dl}       # +1 for the self tile
                for c in range(NCHUNK):
                    _, _, tiles = plan1[g][c]
                    for segs in tiles:
                        for d, _, _ in segs:
                            total_mm[d] += 1

                def psd(d, psmap=psmap):
                    return psmap[d][:]

                for c in range(NCHUNK):
                    t0, seg, tiles = plan1[g][c]
                    if seg == 0:
                        continue
                    gt = gbuf1p.tile([128, max_seg1, 128], BF16, tag="gbuf1")
                    nc.gpsimd.dma_gather(
                        gt[:, :seg, :],
                        x_tab[CHUNK_LO[c]:CHUNK_LO[c] + CHUNK_SIZES[c], :],
                        gix1_s[:, t0 * 8:(t0 + seg) * 8],
                        seg * 128, seg * 128, D, single_packet=False)
                    for ti, segs in enumerate(tiles):
                        s_t = build_s(t0 + ti, colw1_s, nrmw1_s)
                        for d, lo, hi in segs:
                            nmm[d] += 1
                            nc.tensor.matmul(
                                psd(d), gt[lo:hi, ti, :], s_t[lo:hi, :],
                                start=not started[d],
                                stop=nmm[d] == total_mm[d])
                            started[d] = True
                # self tiles: diagonal contribution from own x slab
                sb = stage2p.tile([128, GW * 128], BF16, tag="selfb")
                nc.sync.dma_start(
                    sb[:, :len(dl) * 128],
                    x_self[:, g * GW * D:(g * GW + len(dl)) * D])
                for j, d in enumerate(dl):
                    s_t = build_s(self1[g] + j, colw1_s, nrmw1_s)
                    nmm[d] += 1
                    nc.tensor.matmul(psd(d), sb[:, j * 128:(j + 1) * 128],
                                     s_t[:],
                                     start=not started[d],
                                     stop=nmm[d] == total_mm[d])
                    started[d] = True
                # tail: aggT -> h (batched store), hsT
                h_stage = stage2p.tile([128, GW * 128], BF16, tag="hstage")
                for j, d in enumerate(dl):
                    aggt = stagep.tile([128, 128], BF16, tag="aggt")
                    nc.scalar.activation(aggt[:], psd(d),
                                         mybir.ActivationFunctionType.Copy)
                    ps_h = psop.tile([128, 128], F32, tag="pso")
                    nc.tensor.matmul(ps_h[:], aggt[:], w1_s[:],
                                     start=True, stop=False)
                    nc.tensor.matmul(ps_h[:], ones_s[:], b1_s[:],
                                     start=False, stop=True)
                    h_sb = h_stage[:, j * 128:(j + 1) * 128]
                    nc.scalar.activation(
                        h_sb, ps_h[:], mybir.ActivationFunctionType.Relu)
                    hs = stagep.tile([128, 128], F32, tag="hs")
                    nc.vector.tensor_scalar(
                        hs[:], h_sb, selfw_s[:, d:d + 1], None,
                        mybir.AluOpType.mult)
                    ps_t = pstp.tile([128, 128], F32, tag="pst")
                    nc.tensor.transpose(ps_t[:], hs[:], id_s[:])
                    nc.scalar.activation(
                        hsT_sb[:, d * 128:(d + 1) * 128], ps_t[:],
                        mybir.ActivationFunctionType.Copy)
                hl = h_loc[g * GW * 128:(g * GW + len(dl)) * 128, :]
                hl_p = type(hl)(hl.tensor, hl.offset,
                                [[D, 128], [128 * D, len(dl)], [1, D]])
                nc.sync.dma_start(hl_p, h_stage[:, :len(dl) * 128])
            l1pools.close()

            # ---------------- Layer 2 aggregation + chunked RS ------------
            l2pools = contextlib.ExitStack()
            psag2p = l2pools.enter_context(
                tc.tile_pool(name="psag2", bufs=6, space="PSUM"))
            pso2p = l2pools.enter_context(
                tc.tile_pool(name="pso2", bufs=2, space="PSUM"))
            def tail_chunk(ci):
                col0 = chunk_cols[ci][0]
                reduced = reduceds[ci]
                g_lo = sum(RS_CHUNKS[:ci])
                g_end = g_lo + RS_CHUNKS[ci]
                for g in range(g_lo, g_end):
                    if (g - g_lo) % 4 == 0:
                        nload = min(4, g_end - g)
                        agg2q = stage2p.tile([128, 4 * GW2 * 128], BF16,
                                             tag="agg2")
                        qbase = g * GW2 * 128 - col0
                        qw = 0
                        for gg in range(g, g + nload):
                            qw += len(_group_dtiles2(gg)) * 128
                        nc.sync.dma_start(agg2q[:, :qw],
                                           reduced[:, qbase:qbase + qw])
                        qoff = 0
                    dl = _group_dtiles2(g)
                    agg2 = agg2q[:, qoff:qoff + len(dl) * 128]
                    qoff += len(dl) * 128
                    for j, d in enumerate(dl):
                        ps_o = pso2p.tile([128, 128], F32, tag="pso2")
                        nc.tensor.matmul(
                            ps_o[:], agg2[:, j * 128:(j + 1) * 128], w2_s[:],
                            start=True, stop=False)
                        nc.tensor.matmul(
                            ps_o[:], hsT_sb[:, d * 128:(d + 1) * 128],
                            w2_s[:], start=False, stop=False)
                        nc.tensor.matmul(ps_o[:], ones_s[:], b2_s[:],
                                         start=False, stop=True)
                        o_sb = stagep.tile([128, 128], F32, tag="osb")
                        nc.scalar.activation(
                            o_sb[:], ps_o[:],
                            mybir.ActivationFunctionType.Copy)
                        lo = d * 128
                        hi = min(lo + 128, NPC)
                        nc.sync.dma_start(out_slab[lo:hi, :],
                                           o_sb[:hi - lo, :])

            def emit_rs(ci):
                nc.gpsimd.collective_compute(
                    "ReduceScatter", mybir.AluOpType.add,
                    replica_groups=[list(range(N_CORES))],
                    ins=[partials[ci].opt()], outs=[reduceds[ci].opt()])

            g_hi = 0
            for ci, cgroups in enumerate(RS_CHUNKS):
                g_lo, g_hi = g_hi, g_hi + cgroups
                col0 = chunk_cols[ci][0]
                partial = partials[ci]
                # pair consecutive groups per dc so partial stores batch
                pairs = []
                gg = g_lo
                while gg < g_hi:
                    pairs.append((gg, min(gg + 2, g_hi)))
                    gg += 2
                for pi, (p_lo, p_hi) in enumerate(pairs):
                    if pi == min(3, len(pairs) - 1):
                        # issue the previous chunk's RS now: the first pair's
                        # gathers are already queued on Pool, bridging the
                        # RS dep-wait; its input stores completed by now
                        if ci >= 1:
                            emit_rs(ci - 1)
                        if ci >= 2:
                            tail_chunk(ci - 2)
                    stgs = {}
                    for dc in range(N_CORES):
                        w = sum(len(_group_dtiles2(x)) * 128
                                for x in range(p_lo, p_hi))
                        stg_t = stage2p.tile([128, 2 * GW2 * 128], BF16,
                                             tag=f"pstg{dc}",
                                             name=f"pstg_{dc}")
                        stgs[dc] = (stg_t, w)
                    for g in range(p_lo, p_hi):
                        dl = _group_dtiles2(g)
                        call_base, call_seg, cells = plan2[g]
                        gt = gbuf2p.tile([128, max_seg2, 128], BF16,
                                         tag="gbuf2")
                        if call_seg:
                            nc.gpsimd.dma_gather(
                                gt[:, :call_seg, :], h_loc[:],
                                gix2_s[:,
                                       call_base * 8:(call_base + call_seg) * 8],
                                call_seg * 128, call_seg * 128, D,
                                single_packet=False)
                        goff = (g - p_lo) * GW2 * 128
                        for dc in range(N_CORES):
                            t0, seg, tiles = cells[dc]
                            stg = stgs[dc][0]
                            psmap = {d: psag2p.tile([128, 128], F32,
                                                    tag="psag2",
                                                    name=f"ps2_{g}_{dc}_{d}")
                                     for d in dl}
                            started = {d: False for d in dl}
                            nmm = {d: 0 for d in dl}
                            total_mm = {d: 0 for d in dl}
                            for segs in tiles:
                                for d, _, _ in segs:
                                    total_mm[d] += 1
                            for ti, segs in enumerate(tiles):
                                s_t = build_s(t0 + ti, colw2_s, nrmw2_s)
                                for d, lo, hi in segs:
                                    nmm[d] += 1
                                    nc.tensor.matmul(
                                        psmap[d][:],
                                        gt[lo:hi, t0 - call_base + ti, :],
                                        s_t[lo:hi, :],
                                        start=not started[d],
                                        stop=nmm[d] == total_mm[d])
                                    started[d] = True
                            for j, d in enumerate(dl):
                                dst = stg[:, goff + j * 128:
                                          goff + (j + 1) * 128]
                                if total_mm[d] == 0:
                                    nc.vector.memset(dst, 0.0)
                                else:
                                    nc.scalar.activation(
                                        dst, psmap[d][:],
                                        mybir.ActivationFunctionType.Copy)
                    base = p_lo * GW2 * 128 - col0
                    for dc in range(N_CORES):
                        stg, w = stgs[dc]
                        eng = nc.sync if dc % 2 == 0 else nc.scalar
                        eng.dma_start(
                            partial[dc * 128:(dc + 1) * 128, base:base + w],
                            stg[:, :w])
            nch = len(RS_CHUNKS)
            tail_chunk(nch - 2)
            emit_rs(nch - 1)
            tail_chunk(nch - 1)
            l2pools.close()

    nc.compile()
    return nc


def _preprocess(x, edge_index, edge_weight):
    """Host-side graph preprocessing -> uniform structure + per-core inputs."""
    row = np.asarray(edge_index[0], dtype=np.int64)
    col = np.asarray(edge_index[1], dtype=np.int64)
    ew = np.asarray(edge_weight, dtype=np.float32)

    deg = np.bincount(col, weights=ew.astype(np.float64), minlength=N_NODES)
    deg = (deg + 1.0).astype(np.float32)
    dinv = (1.0 / np.sqrt(deg)).astype(np.float32)
    nrm = (dinv[row] * ew * dinv[col]).astype(np.float32)
    selfw_n = (dinv * dinv).astype(np.float32)

    score = row // NPC
    dcore = col // NPC
    dtile = (col - dcore * NPC) >> 7
    colloc = ((col - dcore * NPC) & 127).astype(np.float32)
    pad_row = (score * PAD_NPC + (row - score * NPC)).astype(np.int64)
    chunk = np.searchsorted(np.asarray(CHUNK_LO), pad_row, side="right") - 1
    grp = dtile // GW

    # ---- layer 1 (keyed by dest core) ----
    key1 = ((dcore * NCHUNK + chunk) * TPC + dtile)
    cnt1 = np.bincount(key1, minlength=N_CORES * NCHUNK * TPC)
    u1 = cnt1.reshape(N_CORES, NCHUNK, TPC).max(axis=0)      # [NCHUNK, TPC]

    NS1, plan1, self1 = _plan_layer1(u1)
    # slot position of (chunk, dtile) span starts (64-padded spans)
    base1 = np.zeros((NCHUNK, TPC), np.int64)
    for g in range(NG):
        for c in range(NCHUNK):
            t0, _, _ = plan1[g][c]
            acc = t0 * 128
            for d in _group_dtiles(g):
                base1[c, d] = acc
                acc += _pad64(u1[c, d])

    order = np.argsort(key1, kind="stable")
    ks = key1[order]
    gstart = np.concatenate(
        [[0], np.cumsum(np.bincount(ks, minlength=key1.max() + 1))[:-1]])
    rank = np.arange(len(ks)) - gstart[ks]

    gix1f = np.zeros((N_CORES, NS1 * 128), np.int16)
    colw1f = np.zeros((N_CORES, NS1 * 128), np.float32)
    nrmw1f = np.zeros((N_CORES, NS1 * 128), np.float32)
    pos = base1[chunk[order], dtile[order]] + rank
    cidx = dcore[order]
    gix1f[cidx, pos] = (pad_row[order]
                        - np.asarray(CHUNK_LO)[chunk[order]]).astype(np.int16)
    colw1f[cidx, pos] = colloc[order]
    nrmw1f[cidx, pos] = nrm[order]
    # self tiles (same for every core's slot space; values differ per core)
    idx_all = np.arange(N_NODES)
    c_all = idx_all // NPC
    selfw_pad = np.zeros(N_CORES * PAD_NPC, np.float32)
    selfw_pad[c_all * PAD_NPC + (idx_all - c_all * NPC)] = selfw_n
    selfw_pc = selfw_pad.reshape(N_CORES, PAD_NPC)
    for g in range(NG):
        for j, d in enumerate(_group_dtiles(g)):
            s = (self1[g] + j) * 128
            colw1f[:, s:s + 128] = np.arange(128, dtype=np.float32)
            nrmw1f[:, s:s + 128] = selfw_pc[:, d * 128:(d + 1) * 128]

    # ---- layer 2 (keyed by source core) ----
    key2 = ((score * N_CORES + dcore) * TPC + dtile)
    cnt2 = np.bincount(key2, minlength=N_CORES * N_CORES * TPC)
    u2 = cnt2.reshape(N_CORES, N_CORES, TPC).max(axis=0)     # [dcore, TPC]

    NT2, plan2 = _plan_layer2(u2)
    base2 = np.zeros((N_CORES, TPC), np.int64)
    for g in range(NG2):
        _, _, cells = plan2[g]
        for dc in range(N_CORES):
            t0, _, _ = cells[dc]
            acc = t0 * 128
            for d in _group_dtiles2(g):
                base2[dc, d] = acc
                acc += _pad64(u2[dc, d])

    order2 = np.argsort(key2, kind="stable")
    ks2 = key2[order2]
    gstart2 = np.concatenate(
        [[0], np.cumsum(np.bincount(ks2, minlength=key2.max() + 1))[:-1]])
    rank2 = np.arange(len(ks2)) - gstart2[ks2]

    gix2f = np.zeros((N_CORES, NT2 * 128), np.int16)
    colw2f = np.zeros((N_CORES, NT2 * 128), np.float32)
    nrmw2f = np.zeros((N_CORES, NT2 * 128), np.float32)
    pos2 = base2[dcore[order2], dtile[order2]] + rank2
    cidx2 = score[order2]
    gix2f[cidx2, pos2] = (row[order2] - score[order2] * NPC).astype(np.int16)
    colw2f[cidx2, pos2] = colloc[order2]
    nrmw2f[cidx2, pos2] = nrm[order2]

    def wrap_gix(gixf, nt):
        g = gixf.reshape(N_CORES, nt * 8, 16).transpose(0, 2, 1)
        return np.ascontiguousarray(np.tile(g, (1, 8, 1)))

    def wrap_col(f, nt):
        return np.ascontiguousarray(
            f.reshape(N_CORES, nt, 128).transpose(0, 2, 1))

    gix1w = wrap_gix(gix1f, NS1)
    colw1w = wrap_col(colw1f, NS1)
    nrmw1w = wrap_col(nrmw1f, NS1)
    gix2w = wrap_gix(gix2f, NT2)
    colw2w = wrap_col(colw2f, NT2)
    nrmw2w = wrap_col(nrmw2f, NT2)

    selfw_t = np.ascontiguousarray(
        selfw_pad.reshape(N_CORES, TPC, 128).transpose(0, 2, 1))

    x = np.asarray(x, dtype=np.float32)
    x_tab = np.zeros((N_PAD, D), NPBF)
    x_tab.reshape(N_CORES, PAD_NPC, D)[:, :NPC, :] = \
        x.reshape(N_CORES, NPC, D).astype(NPBF)
    # per-core x slab permuted to [row-within-dtile, dtile*D + f]
    x_selfs = np.ascontiguousarray(
        x_tab.reshape(N_CORES, TPC, 128, D).transpose(0, 2, 1, 3)
        .reshape(N_CORES, 128, TPC * D))

    u1_key = tuple(int(v) for v in u1.reshape(-1))
    u2_key = tuple(int(v) for v in u2.reshape(-1))
    return (u1_key, u2_key, gix1w, colw1w, nrmw1w, gix2w, colw2w, nrmw2w,
            selfw_t, x_tab, x_selfs)


_NC_CACHE: dict = {}


def kernel(x, edge_index, edge_weight, W1, b1, W2, b2):
    (u1_key, u2_key, gix1w, colw1w, nrmw1w, gix2w, colw2w, nrmw2w,
     selfw_t, x_tab, x_selfs) = _preprocess(x, edge_index, edge_weight)

    ck = (u1_key, u2_key)
    if ck not in _NC_CACHE:
        _NC_CACHE[ck] = _build_nc(u1_key, u2_key)
    nc = _NC_CACHE[ck]

    w1_np = np.ascontiguousarray(np.asarray(W1, dtype=np.float32).astype(NPBF))
    w2_np = np.ascontiguousarray(np.asarray(W2, dtype=np.float32).astype(NPBF))
    b1_np = np.asarray(b1, dtype=np.float32).astype(NPBF).reshape(1, D)
    b2_np = np.asarray(b2, dtype=np.float32).astype(NPBF).reshape(1, D)
    ident = np.eye(128, dtype=np.float32)
    iota = np.tile(np.arange(128), (128, 1)).astype(NPBF)
    ones = np.ones((1, D), NPBF)

    in_maps = []
    for c in range(N_CORES):
        in_maps.append({
            "x_tab": x_tab, "x_self": x_selfs[c],
            "gix1": gix1w[c], "colw1": colw1w[c], "nrmw1": nrmw1w[c],
            "gix2": gix2w[c], "colw2": colw2w[c], "nrmw2": nrmw2w[c],
            "selfw": selfw_t[c],
            "w1": w1_np, "b1": b1_np, "w2": w2_np, "b2": b2_np,
            "ident": ident, "iota": iota, "ones": ones,
        })

    res = run_bass_kernel_spmd(nc, in_maps, core_ids=list(range(N_CORES)))
    out = np.concatenate([res.results[c]["out_slab"] for c in range(N_CORES)],
                         axis=0)
    return out
